# revision 16
# baseline (speedup 1.0000x reference)
"""Distributed Trainium2 kernel for the dense transformer block.

Strategy (8 NeuronCores, SPMD), v3 — (kv-group x batch)-parallel attention:
  Core c handles query group g = c//2 of batch beta = c%2 for ALL 2048
  tokens. qkv projection (fp8 DoubleRow matmuls), depthwise causal conv,
  SiLU and RoPE are fully local (no halo, no collective). Causal
  attention for the core's 4 heads is fully local: scores in bf16, exp
  written straight to fp8, AV as fp8 DoubleRow, rowsum as narrow (M=4)
  DoubleRow matmuls. Two fp8 AllToAlls (heads 01 / 23) reshard y to
  token-parallel (256 tokens of each batch per core); the proj
  accumulation is split even/odd so the second A2A hides behind the
  first half of proj.
  Phase C: proj fp8 DR + residual -> rmsnorm2 -> gated MLP in bf16
  (fp8 fails the precision budget there) -> residual.

All fp8 operands use a fixed scale of 32 (values < 6 in magnitude;
fp8e4 clips at 240). Weights are quantized per output channel on the
host; dequant scales fold into the PSUM->SBUF copies.
"""
import os
import sys

sys.path.insert(0, "/opt/trn_rl_repo")

import numpy as np
import ml_dtypes

import concourse.bass as bass
import concourse.mybir as mybir
from concourse import bacc, tile
from concourse.bass_utils import run_bass_kernel_spmd

B, T, C = 2, 2048, 2048
NH, NG, HS = 16, 4, 128
QPK = NH // NG
DCONV = 4
IM = 5632
EPS = 1e-5
NCORES = 8
NKC = C // 128       # 16
NMI = IM // 128      # 44
TOK = 512            # phase-C tokens per core (256 of each batch)
SCALE = 1.0 / float(np.sqrt(HS))
QS = 32.0            # fp8 activation scale
LN_QS = float(np.log(QS))

F32 = mybir.dt.float32
BF16 = mybir.dt.bfloat16
FP8 = mybir.dt.float8e4
AF = mybir.ActivationFunctionType
ALU = mybir.AluOpType
PM = mybir.MatmulPerfMode

DEBUG = bool(int(os.environ.get("KERNEL_DEBUG", "0")))
TRACE = bool(int(os.environ.get("KERNEL_TRACE", "0")))

LAST_RESULTS = None  # test.py reads exec_time from here


# --------------------------------------------------------------------------
# builder
# --------------------------------------------------------------------------

def build_nc():
    nc = bacc.Bacc("TRN2", target_bir_lowering=False, debug=False,
                   enable_asserts=True, num_devices=NCORES)

    # per-core inputs
    x8_d = nc.dram_tensor("x8", [128, NKC, T], FP8, kind="ExternalInput")
    xc_d = nc.dram_tensor("xc", [C, TOK], F32, kind="ExternalInput")
    wq_d = nc.dram_tensor("wq", [6, 128, NKC, 128], FP8, kind="ExternalInput")
    tmq_d = nc.dram_tensor("tmq", [128, 6], F32, kind="ExternalInput")
    wp_d = nc.dram_tensor("wp", [16, 128, NKC, 128], FP8, kind="ExternalInput")
    tmp_d = nc.dram_tensor("tmp", [128, 16], F32, kind="ExternalInput")
    w1_d = nc.dram_tensor("w1", [NMI, 128, C], BF16, kind="ExternalInput")
    w2_d = nc.dram_tensor("w2", [NMI, 128, C], BF16, kind="ExternalInput")
    wm_d = nc.dram_tensor("wm", [16, 128, IM], BF16, kind="ExternalInput")
    cw_d = nc.dram_tensor("cw", [128, 6 * DCONV], F32, kind="ExternalInput")
    trig_d = nc.dram_tensor("trig", [128, 2 * T], BF16, kind="ExternalInput")
    msk_d = nc.dram_tensor("msk", [128, 2048], BF16, kind="ExternalInput")
    rotm_d = nc.dram_tensor("rotm", [128, 128], BF16, kind="ExternalInput")
    out_d = nc.dram_tensor("out", [C, TOK], F32, kind="ExternalOutput")

    # collective buffers: chunk j goes to / comes from core j.
    # dim1 rows: head pair (0,1) for a=0, (2,3) for a=1.
    t2i = [nc.dram_tensor(f"t2i{a}", [NCORES, 256, 256], BF16, kind="Internal")
           for a in range(2)]
    t2o = [nc.dram_tensor(f"t2o{a}", [NCORES, 256, 256], BF16, kind="Internal")
           for a in range(2)]

    dbg = {}
    if DEBUG:
        dbg["sl"] = nc.dram_tensor("d_sl", [6 * 128, T], BF16, kind="ExternalOutput")
        dbg["y8"] = nc.dram_tensor("d_y8", [QPK * 128, T], BF16, kind="ExternalOutput")
        dbg["x2"] = nc.dram_tensor("d_x2", [C, TOK], F32, kind="ExternalOutput")
        dbg["rinv"] = nc.dram_tensor("d_rinv", [1, T], F32, kind="ExternalOutput")

    with tile.TileContext(nc) as tc:
        with tc.tile_pool(name="pers", bufs=1) as pers:
            # ---- constants ----
            cw_sb = pers.tile([128, 6 * DCONV], F32, tag="cw", name="cw")
            tmq_sb = pers.tile([128, 6], F32, tag="tmq", name="tmq")
            tmp_sb = pers.tile([128, 16], F32, tag="tmp", name="tmp")
            nc.sync.dma_start(cw_sb[:], cw_d[:])
            nc.sync.dma_start(tmq_sb[:], tmq_d[:])
            nc.sync.dma_start(tmp_sb[:], tmp_d[:])

            ones128 = pers.tile([128, 128], BF16, tag="ones128", name="ones128")
            ones8 = pers.tile([128, 2, 32], FP8, tag="ones8", name="ones8")
            eps1 = pers.tile([1, 1], F32, tag="eps1", name="eps1")
            lnq = pers.tile([128, 1], F32, tag="lnq", name="lnq")
            nc.gpsimd.memset(ones128[:], 1.0)
            nc.gpsimd.memset(ones8[:], 1.0)
            nc.gpsimd.memset(eps1[:], EPS)
            nc.gpsimd.memset(lnq[:], LN_QS)

            # pool spanning phases A+B (closed before C to free SBUF)
            pab_cm = tc.tile_pool(name="pab_sb", bufs=1)
            pab = pab_cm.__enter__()
            msk_sb = pab.tile([128, 2048], BF16, tag="msk", name="msk")
            nc.sync.dma_start(msk_sb[:], msk_d[:])
            qall = [pab.tile([128, T], BF16, tag=f"q{h}", name=f"q{h}")
                    for h in range(QPK)]
            kall = pab.tile([128, T], BF16, tag="kall", name="kall")
            v8t = pab.tile([128, NKC, 128], FP8, tag="v8t", name="v8t")
            y8 = [pab.tile([128, T], BF16, tag=f"y8_{h}", name=f"y8_{h}")
                  for h in range(QPK)]

            # ========================================================
            # Phases A+B interleaved at head granularity
            # ========================================================
            with tc.tile_pool(name="pa_sb", bufs=1) as pa, \
                 tc.tile_pool(name="pab_ps", bufs=1, space="PSUM") as pap, \
                 tc.tile_pool(name="pb_sb", bufs=1) as pb:
                pbp = pap
                trig_sb = pa.tile([128, 2 * T], BF16, tag="trig", name="trig")
                nc.sync.dma_start(trig_sb[:], trig_d[:])
                x8 = pa.tile([128, NKC, T], FP8, tag="x8", name="x8")
                for qa in range(4):
                    nc.sync.dma_start(x8[:, qa * 4:(qa + 1) * 4, :],
                                      x8_d[:, qa * 4:(qa + 1) * 4, :])
                wq_sb = [pa.tile([128, NKC, 128], FP8, tag=f"wq{m}", name=f"wq{m}")
                         for m in range(6)]
                for m in range(6):
                    nc.sync.dma_start(wq_sb[m][:], wq_d[m])

                # ---- rmsnorm scale (tq-major; squares split Act/DVE) ----
                rinvb = pa.tile([128, T], F32, tag="rinvb", name="rinvb")
                for tq in range(4):
                    sl512 = slice(tq * 512, (tq + 1) * 512)
                    ss_ps = pap.tile([128, 512], F32, tag="qk", bufs=2, name="ss")
                    for kk in range(NKC):
                        xsq = pa.tile([128, 512], BF16, tag="xsq", bufs=3, name="xsq")
                        if kk % 2 == 0:
                            nc.scalar.activation(xsq[:], x8[:, kk, sl512], AF.Square)
                        else:
                            nc.vector.tensor_mul(xsq[:], x8[:, kk, sl512],
                                                 x8[:, kk, sl512])
                        nc.tensor.matmul(ss_ps[:], ones128[:], xsq[:],
                                         start=(kk == 0), stop=(kk == NKC - 1))
                    rt = pa.tile([1, 512], F32, tag="rt", bufs=2, name="rt")
                    nc.scalar.activation(rt[:], ss_ps[0:1, :], AF.Sqrt,
                                         bias=eps1[:], scale=1.0 / (C * QS * QS))
                    rinv = pa.tile([1, 512], F32, tag="rinv", bufs=2, name="rinv")
                    nc.vector.reciprocal(rinv[:], rt[:])
                    nc.gpsimd.partition_broadcast(rinvb[:, sl512], rinv[:])
                if DEBUG:
                    nc.sync.dma_start(dbg["rinv"][:], rinvb[0:1, :])

                pre = {}

                def qkv_mtile(m):
                    """qkv DR matmuls + dequant for local m-tile."""
                    pre[m] = pa.tile([128, DCONV - 1 + T], BF16,
                                     tag="pre", bufs=3, name=f"pre{m}")
                    nc.gpsimd.memset(pre[m][:, 0:DCONV - 1], 0.0)
                    for tq in range(4):
                        sl512 = slice(tq * 512, (tq + 1) * 512)
                        qk_ps = pap.tile([128, 512], F32, tag="qk", bufs=2,
                                         name="qk")
                        for jp in range(NKC // 2):
                            nc.tensor.matmul(
                                qk_ps[:], wq_sb[m][:, 2 * jp:2 * jp + 2, :],
                                x8[:, 2 * jp:2 * jp + 2, sl512],
                                start=(jp == 0), stop=(jp == NKC // 2 - 1),
                                perf_mode=PM.DoubleRow)
                        nc.vector.scalar_tensor_tensor(
                            pre[m][:, DCONV - 1 + tq * 512:DCONV - 1 + (tq + 1) * 512],
                            qk_ps[:], tmq_sb[:, m:m + 1], rinvb[:, sl512],
                            op0=ALU.mult, op1=ALU.mult)

                def convrope_mtile(m):
                    """causal conv + silu (+rope for q/k, fp8 transpose for v).
                    Full-row (2048-wide) ops to amortize DVE instruction
                    overhead."""
                    acc = pa.tile([128, T], F32, tag="cacc", bufs=1, name="cacc")
                    nc.vector.tensor_scalar_mul(acc[:], pre[m][:, 0:T],
                                                cw_sb[:, m * 4:m * 4 + 1])
                    for j in range(1, DCONV):
                        nc.vector.scalar_tensor_tensor(
                            acc[:], pre[m][:, j:j + T],
                            cw_sb[:, m * 4 + j:m * 4 + j + 1], acc[:],
                            op0=ALU.mult, op1=ALU.add)
                    if m < 5:
                        sl = pa.tile([128, T], BF16, tag="sl", bufs=1, name="sl")
                        nc.scalar.activation(sl[:], acc[:], AF.Silu)
                        if DEBUG:
                            nc.sync.dma_start(dbg["sl"][m * 128:(m + 1) * 128, :],
                                              sl[:])
                        dst = qall[m][:] if m < 4 else kall[:]
                        # rot = [x2; x1] via partition-swap DMAs; the sign for
                        # the rotate-half lives in the sin table
                        rot_sb = pa.tile([128, T], BF16, tag="rsb", bufs=1,
                                         name="rsb")
                        nc.sync.dma_start(rot_sb[0:64, :], sl[64:128, :])
                        nc.sync.dma_start(rot_sb[64:128, :], sl[0:64, :])
                        tt1 = pa.tile([128, T], BF16, tag="tt1", bufs=1,
                                      name="tt1")
                        nc.vector.tensor_mul(tt1[:], sl[:], trig_sb[:, 0:T])
                        tt2 = pa.tile([128, T], BF16, tag="tt2", bufs=1,
                                      name="tt2")
                        nc.vector.tensor_mul(tt2[:], rot_sb[:], trig_sb[:, T:2 * T])
                        nc.vector.tensor_add(dst[:], tt1[:], tt2[:])
                    else:
                        vsl = pa.tile([128, T], BF16, tag="vsl", bufs=1, name="vsl")
                        nc.scalar.activation(vsl[:], acc[:], AF.Silu)
                        if DEBUG:
                            nc.sync.dma_start(dbg["sl"][5 * 128:6 * 128, :], vsl[:])
                        vt = pa.tile([128, NKC, 128], BF16, tag="vt", bufs=1,
                                     name="vt")
                        for i in range(NKC):
                            nc.sync.dma_start_transpose(
                                vt[:, i, :], vsl[:, i * 128:(i + 1) * 128])
                        with nc.allow_low_precision(reason="fp8 v"):
                            nc.scalar.activation(v8t[:], vt[:], AF.Copy, scale=QS)

                def attn_head(h):
                    """causal attention for local head h -> y8[h]."""
                    for bp in range(4):
                        npair = 2 * (bp + 1)
                        # diagonal pairs first so the AV tail never waits on
                        # the mask multiply
                        order = [npair - 2, npair - 1] + list(range(npair - 2))
                        o_ps = pbp.tile([128, 512], F32, tag="o", bufs=1, name="o")
                        rs_ps = pbp.tile([32, 512], F32, tag="rs", bufs=1, name="rs")
                        p8s = {}
                        nav = [0]

                        def av_pair(jp):
                            nc.tensor.matmul(
                                o_ps[:], v8t[:, 2 * jp:2 * jp + 2, :], p8s[jp][:],
                                start=(nav[0] == 0), stop=(nav[0] == npair - 1),
                                perf_mode=PM.DoubleRow)
                            nc.tensor.matmul(
                                rs_ps[:], ones8[:], p8s[jp][:],
                                start=(nav[0] == 0), stop=(nav[0] == npair - 1),
                                perf_mode=PM.DoubleRow)
                            nav[0] += 1

                        for idx, jp in enumerate(order):
                            s_ps = pbp.tile([128, 2, 512], F32, tag="s", bufs=2,
                                            name="s")
                            for i in range(2):
                                nc.tensor.matmul(
                                    s_ps[:, i, :],
                                    kall[:, (jp * 2 + i) * 128:(jp * 2 + i + 1) * 128],
                                    qall[h][:, bp * 512:(bp + 1) * 512],
                                    start=True, stop=True)
                            p8 = pb.tile([128, 2, 512], FP8, tag="p8", bufs=5,
                                         name="p8")
                            p8s[jp] = p8
                            with nc.allow_low_precision(reason="fp8 probs"):
                                if idx < 2:  # diagonal pair: mask
                                    pd = pb.tile([128, 2, 512], BF16, tag="pd",
                                                 bufs=2, name="pd")
                                    nc.scalar.activation(pd[:], s_ps[:], AF.Exp,
                                                         bias=lnq[:], scale=SCALE)
                                    mof = 0 if idx == 0 else 1024
                                    nc.vector.tensor_mul(
                                        p8[:].rearrange("p a b -> p (a b)"),
                                        pd[:].rearrange("p a b -> p (a b)"),
                                        msk_sb[:, mof:mof + 1024])
                                else:
                                    nc.scalar.activation(p8[:], s_ps[:], AF.Exp,
                                                         bias=lnq[:], scale=SCALE)
                            if idx >= 2:
                                av_pair(order[idx - 2])
                        av_pair(order[npair - 2])
                        av_pair(order[npair - 1])
                        rho = pb.tile([1, 512], F32, tag="rho", bufs=2, name="rho")
                        nc.vector.reciprocal(rho[:], rs_ps[0:1, :])
                        rhob = pb.tile([128, 512], F32, tag="rhob", bufs=2,
                                       name="rhob")
                        nc.gpsimd.partition_broadcast(rhob[:], rho[:])
                        with nc.allow_low_precision(reason="bf16 y"):
                            nc.vector.tensor_mul(
                                y8[h][:, bp * 512:(bp + 1) * 512], o_ps[:], rhob[:])
                    if DEBUG:
                        nc.sync.dma_start(dbg["y8"][h * 128:(h + 1) * 128, :],
                                          y8[h][:])

                def fire_a2a(a):
                    """AllToAll for head pair (2a, 2a+1)."""
                    for hh in range(2):
                        for j in range(NCORES):
                            nc.sync.dma_start(
                                t2i[a][j, hh * 128:(hh + 1) * 128, :],
                                y8[2 * a + hh][:, 256 * j:256 * (j + 1)])
                    nc.gpsimd.collective_compute(
                        "AllToAll", ALU.bypass,
                        replica_groups=[list(range(NCORES))],
                        ins=[t2i[a][:].opt()], outs=[t2o[a][:].opt()])

                # ---- emission schedule: 1-mtile software pipeline so
                # PE (qkv m) overlaps DVE/Act (conv/rope m-1, attention) ----
                qkv_mtile(4)            # k
                qkv_mtile(5)            # v
                convrope_mtile(4)
                qkv_mtile(0)
                convrope_mtile(5)
                qkv_mtile(1)
                convrope_mtile(0)
                attn_head(0)
                qkv_mtile(2)
                convrope_mtile(1)
                attn_head(1)
                fire_a2a(0)
                qkv_mtile(3)
                convrope_mtile(2)
                attn_head(2)
                convrope_mtile(3)
                attn_head(3)
                fire_a2a(1)

            pab_cm.__exit__(None, None, None)

            # ============================================================
            # Phase C: proj (fp8 DR, even/odd split) + residual, norm2,
            #          MLP (bf16), output
            # ============================================================
            with tc.tile_pool(name="pc_sb", bufs=1) as pc_, \
                 tc.tile_pool(name="pc_ps", bufs=1, space="PSUM") as pcp:
                x2 = [pc_.tile([128, TOK], F32, tag=f"x2_{i}", name=f"x2_{i}")
                      for i in range(NKC)]
                n2 = [pc_.tile([128, TOK], BF16, tag=f"n2_{i}", name=f"n2_{i}")
                      for i in range(NKC)]
                h_t = [pc_.tile([128, TOK], BF16, tag=f"h{i}", name=f"h{i}")
                       for i in range(NMI)]

                with tc.tile_pool(name="pc0", bufs=1) as pc0:
                    # order matters: nothing A2A2-dependent may sit ahead of
                    # the proj-evens inputs in the DMA queues
                    wp_sb = [pc0.tile([128, NKC, 128], FP8, tag=f"wp{mo}",
                                      name=f"wp{mo}") for mo in range(16)]
                    for mo in range(16):
                        nc.sync.dma_start(wp_sb[mo][:], wp_d[mo])
                    xc = [pc0.tile([128, TOK], F32, tag=f"xc{i}", name=f"xc{i}")
                          for i in range(NKC)]
                    for kk in range(NKC):
                        nc.sync.dma_start(xc[kk][:], xc_d[kk * 128:(kk + 1) * 128, :])
                    # gather y (bf16) and cast to fp8: ysb[p, kk, tok],
                    # kk = g'*4 + h; head-01 slices (first A2A) first
                    ysbb = pc0.tile([128, NKC, TOK], BF16, tag="ysbb", name="ysbb")
                    ysb = pc0.tile([128, NKC, TOK], FP8, tag="ysb", name="ysb")
                    for a in range(2):
                        for hh in range(2):
                            for gp in range(4):
                                for b in range(2):
                                    nc.sync.dma_start(
                                        ysbb[:, gp * 4 + 2 * a + hh,
                                             b * 256:(b + 1) * 256],
                                        t2o[a][2 * gp + b,
                                               hh * 128:(hh + 1) * 128, :])
                        with nc.allow_low_precision(reason="fp8 y cast"):
                            for gp in range(4):
                                kk0 = gp * 4 + 2 * a
                                nc.scalar.activation(
                                    ysb[:, kk0:kk0 + 2, :], ysbb[:, kk0:kk0 + 2, :],
                                    AF.Copy)
                    # proj: per block, accumulate head-01 pairs (jp even,
                    # first A2A) then head-23 pairs (jp odd, second A2A)
                    evens = [0, 2, 4, 6]
                    odds = [1, 3, 5, 7]
                    for blk in (range(0, 6), range(6, 11), range(11, 16)):
                        mm_tiles = {}
                        for mo in blk:
                            mm_ps = pcp.tile([128, TOK], F32, tag="mm", bufs=6,
                                             name="mm")
                            mm_tiles[mo] = mm_ps
                            for ij, jp in enumerate(evens):
                                nc.tensor.matmul(
                                    mm_ps[:], wp_sb[mo][:, 2 * jp:2 * jp + 2, :],
                                    ysb[:, 2 * jp:2 * jp + 2, :],
                                    start=(ij == 0), stop=False,
                                    perf_mode=PM.DoubleRow)
                        for mo in blk:
                            for ij, jp in enumerate(odds):
                                nc.tensor.matmul(
                                    mm_tiles[mo][:],
                                    wp_sb[mo][:, 2 * jp:2 * jp + 2, :],
                                    ysb[:, 2 * jp:2 * jp + 2, :],
                                    start=False, stop=(ij == len(odds) - 1),
                                    perf_mode=PM.DoubleRow)
                            nc.vector.scalar_tensor_tensor(
                                x2[mo][:], mm_tiles[mo][:], tmp_sb[:, mo:mo + 1],
                                xc[mo][:], op0=ALU.mult, op1=ALU.add)
                            if DEBUG:
                                nc.sync.dma_start(
                                    dbg["x2"][mo * 128:(mo + 1) * 128, :], x2[mo][:])

                ss2 = pcp.tile([128, TOK], F32, tag="nrm", bufs=2, name="nrm")
                for kk in range(NKC):
                    x2sq = pc_.tile([128, TOK], BF16, tag="x2sq", bufs=3, name="x2sq")
                    nc.scalar.activation(x2sq[:], x2[kk][:], AF.Square)
                    nc.tensor.matmul(ss2[:], ones128[:], x2sq[:],
                                     start=(kk == 0), stop=(kk == NKC - 1))
                rt2 = pc_.tile([1, TOK], F32, tag="rt2", bufs=1, name="rt2")
                nc.scalar.activation(rt2[:], ss2[0:1, :], AF.Sqrt, bias=eps1[:],
                                     scale=1.0 / C)
                rinv2 = pc_.tile([1, TOK], F32, tag="rinv2", bufs=1, name="rinv2")
                nc.vector.reciprocal(rinv2[:], rt2[:])
                rb2 = pc_.tile([128, TOK], F32, tag="rb2", bufs=1, name="rb2")
                nc.gpsimd.partition_broadcast(rb2[:], rinv2[:])
                for kk in range(NKC):
                    nc.vector.tensor_mul(n2[kk][:], x2[kk][:], rb2[:])

                for mi in range(NMI):
                    w1_sb = pc_.tile([128, C], BF16, tag="wst", bufs=3, name="wst")
                    nc.sync.dma_start(w1_sb[:], w1_d[mi])
                    h1_ps = pcp.tile([128, TOK], F32, tag="mm", bufs=6, name="mm")
                    for kk in range(NKC):
                        nc.tensor.matmul(h1_ps[:],
                                         w1_sb[:, kk * 128:(kk + 1) * 128],
                                         n2[kk][:],
                                         start=(kk == 0), stop=(kk == NKC - 1))
                    s1 = pc_.tile([128, TOK], BF16, tag="s1", bufs=2, name="s1")
                    nc.scalar.activation(s1[:], h1_ps[:], AF.Silu)
                    w2_sb = pc_.tile([128, C], BF16, tag="wst", bufs=3, name="wst")
                    nc.sync.dma_start(w2_sb[:], w2_d[mi])
                    h2_ps = pcp.tile([128, TOK], F32, tag="mm", bufs=6, name="mm")
                    for kk in range(NKC):
                        nc.tensor.matmul(h2_ps[:],
                                         w2_sb[:, kk * 128:(kk + 1) * 128],
                                         n2[kk][:],
                                         start=(kk == 0), stop=(kk == NKC - 1))
                    nc.vector.tensor_mul(h_t[mi][:], s1[:], h2_ps[:])

                with tc.tile_pool(name="pcm", bufs=1) as pcm:
                    for mo in range(16):
                        wm_sb = pcm.tile([128, IM], BF16, tag="wm", bufs=2, name="wm")
                        nc.sync.dma_start(wm_sb[:], wm_d[mo])
                        mp_ps = pcp.tile([128, TOK], F32, tag="mm", bufs=6, name="mm")
                        for ki in range(NMI):
                            nc.tensor.matmul(mp_ps[:],
                                             wm_sb[:, ki * 128:(ki + 1) * 128],
                                             h_t[ki][:],
                                             start=(ki == 0), stop=(ki == NMI - 1))
                        outsb = pc_.tile([128, TOK], F32, tag="outsb", bufs=2,
                                         name="outsb")
                        nc.vector.tensor_add(outsb[:], x2[mo][:], mp_ps[:])
                        nc.sync.dma_start(out_d[mo * 128:(mo + 1) * 128, :], outsb[:])

    nc.compile()
    return nc


# --------------------------------------------------------------------------
# host-side prep / gather
# --------------------------------------------------------------------------

def _q8(a):
    return np.clip(a, -240.0, 240.0).astype(ml_dtypes.float8_e4m3)


def _prep_fp8_lhsT(w, nm, nk):
    """w: (out, in) f32 -> (lhsT fp8 [nm,128,nk,128], scales f32 [128,nm])
    with per-output-channel absmax quantization. Dequant scale includes
    the 1/QS for the fp8 rhs activations."""
    o, i = w.shape
    assert o == nm * 128 and i == nk * 128
    r = w.reshape(nm, 128, nk, 128).transpose(0, 3, 2, 1)  # (m, p, k, c)
    amax = np.abs(r).max(axis=(1, 2))                      # (m, c)
    amax = np.maximum(amax, 1e-30)
    q = _q8(r * (240.0 / amax[:, None, None, :]))
    scales = np.ascontiguousarray((amax / (240.0 * QS)).T).astype(np.float32)
    return np.ascontiguousarray(q), scales


def _prep_lhsT(w, nm, nk):
    """w: (out, in) f32 -> (nm, 128, nk*128) bf16 where
    prep[m][p][k*128+c] = w[m*128+c, k*128+p]."""
    o, i = w.shape
    assert o == nm * 128 and i == nk * 128
    r = w.reshape(nm, 128, nk, 128).transpose(0, 3, 2, 1)
    return np.ascontiguousarray(r.reshape(nm, 128, nk * 128)).astype(ml_dtypes.bfloat16)


def _host_inputs(inputs):
    x = np.asarray(inputs["x"], np.float32)          # (B, T, C)
    cos = np.asarray(inputs["cos"], np.float32)      # (T, 64)
    sin = np.asarray(inputs["sin"], np.float32)
    n1w = np.asarray(inputs["norm1_w"], np.float32)
    n2w = np.asarray(inputs["norm2_w"], np.float32)

    attn_w = np.asarray(inputs["attn_w"], np.float32) * n1w[None, :]
    fc1_w = np.asarray(inputs["fc1_w"], np.float32) * n2w[None, :]
    fc2_w = np.asarray(inputs["fc2_w"], np.float32) * n2w[None, :]
    proj_w = np.asarray(inputs["proj_w"], np.float32)
    mlp_w = np.asarray(inputs["mlp_proj_w"], np.float32)

    wq_all, tmq_all = _prep_fp8_lhsT(attn_w, NH + 2 * NG, NKC)  # (24,128,16,128)
    wp, tmp_s = _prep_fp8_lhsT(proj_w, 16, NKC)
    w1 = _prep_lhsT(fc1_w, NMI, NKC)
    w2 = _prep_lhsT(fc2_w, NMI, NKC)
    wm = _prep_lhsT(mlp_w, 16, NMI)

    qc = np.asarray(inputs["qconv_w"], np.float32)
    kc = np.asarray(inputs["kconv_w"], np.float32)
    vc = np.asarray(inputs["vconv_w"], np.float32)

    # masks: mskA for pair npair-2 (tk rel = i*128+p), mskB for npair-1
    p = np.arange(128)[:, None]
    f = np.arange(512)[None, :]
    mskA = np.concatenate([(p <= f), (p + 128 <= f)], axis=1)
    mskB = np.concatenate([(p + 256 <= f), (p + 384 <= f)], axis=1)
    msk = np.concatenate([mskA, mskB], axis=1).astype(np.float32)
    msk = msk.astype(ml_dtypes.bfloat16)

    rotm = np.zeros((128, 128), np.float32)
    for m in range(64):
        rotm[m + 64, m] = -1.0
        rotm[m, m + 64] = 1.0
    rotm = rotm.astype(ml_dtypes.bfloat16)

    # trig [128, 2T]: cols 0:T cos (64-halves stacked), T:2T sin
    cosT = cos.T                                     # (64, T)
    sinT = sin.T
    cs = np.concatenate([cosT, cosT], axis=0)        # (128, T)
    ss = np.concatenate([-sinT, sinT], axis=0)       # rotate-half sign folded
    trig = np.ascontiguousarray(
        np.concatenate([cs, ss], axis=1)).astype(ml_dtypes.bfloat16)

    # x8 per batch: [128, NKC, T] with x8[p,kk,t] = q8(32*x[beta,t,kk*128+p])
    xt = x.transpose(0, 2, 1)                        # (B, C, T)
    x8b = []
    for beta in range(B):
        a = xt[beta].reshape(NKC, 128, T).transpose(1, 0, 2)  # (128, NKC, T)
        x8b.append(np.ascontiguousarray(_q8(a * QS)))

    in_maps = []
    for c in range(NCORES):
        g, beta = c // 2, c % 2
        msel = [g * 6 + s for s in range(6)]
        wq = np.ascontiguousarray(wq_all[msel])
        tmq = np.ascontiguousarray(tmq_all[:, msel])
        cw = np.zeros((128, 6 * DCONV), np.float32)
        for s in range(QPK):
            cw[:, s * DCONV:(s + 1) * DCONV] = qc[(g * QPK + s) * 128:(g * QPK + s + 1) * 128]
        cw[:, 4 * DCONV:5 * DCONV] = kc[g * 128:(g + 1) * 128]
        cw[:, 5 * DCONV:6 * DCONV] = vc[g * 128:(g + 1) * 128]

        # phase-C residual x: feature-major, cols = [b0 tokens | b1 tokens]
        xc = np.zeros((C, TOK), np.float32)
        for b in range(B):
            xc[:, b * 256:(b + 1) * 256] = xt[b][:, 256 * c:256 * (c + 1)]

        in_maps.append({
            "x8": x8b[beta], "xc": xc, "wq": wq, "tmq": tmq,
            "wp": wp, "tmp": tmp_s, "w1": w1, "w2": w2, "wm": wm,
            "cw": cw, "trig": trig, "msk": msk, "rotm": rotm,
        })
    return in_maps


_NC_CACHE = None


def kernel(**inputs) -> np.ndarray:
    global LAST_RESULTS, _NC_CACHE
    if _NC_CACHE is None:
        _NC_CACHE = build_nc()
    nc = _NC_CACHE
    in_maps = _host_inputs(inputs)
    res = run_bass_kernel_spmd(nc, in_maps, list(range(NCORES)), trace=TRACE)
    LAST_RESULTS = res
    out = np.zeros((B, T, C), np.float32)
    for c in range(NCORES):
        oc = res.results[c]["out"]                   # (C, TOK) feature-major
        for b in range(B):
            out[b, 256 * c:256 * (c + 1), :] = oc[:, b * 256:(b + 1) * 256].T
    return out
